# revision 34
# baseline (speedup 1.0000x reference)
"""MoE expert-routing kernel for Trainium2 (8 NeuronCores, expert-parallel).

Problem: out[t] = x[t] @ weight[index[t]] + bias[index[t]]
  x: (32768, 512) f32, index: (32768,) int, weight: (8, 512, 512) f32,
  bias: (8, 512) f32.

Strategy (expert-parallel, host-side dispatch):
  Core e owns expert e. The host gathers the tokens routed to expert e
  into a fixed-capacity, transposed buffer xt_e[512, CAP] (padded with
  zeros), and core e computes y_e = x_e @ W_e + b_e as a single dense
  GEMM. Results are scattered back to token order on the host. Tokens
  beyond CAP fall back to a host matmul, so the kernel stays correct
  for any index distribution.

Device kernel (per core): y = x_e @ W_e + b_e over CAP=4224 tokens
  - Host packs x_e pre-transposed AND slab-contiguous (single
    contiguous run per partition per slab DMA; no strided descriptors).
  - Measurement model (from NTFF traces): exec_time_ns counts from the
    framework's first constant-memset (~5.9us; the runtime-go wait and
    engine barriers before it are EXCLUDED) to the end of the last
    instruction, which includes a runtime-injected ~6.7us postamble
    zeroing the 256-entry semaphore file one EVENT_SEMAPHORE per sem
    across 5 engines. Both ends are fixed framework cost; the
    optimizable window is user work (~7.2us to last-DMA-receipt).
  - Jitter model (v2-v4 experiments): DMA completion-semaphore
    visibility at the consumer engine lags the semaphore update by
    0.4-2.8us, run-dependent (worst at startup, when all 8 cores
    stream their heads simultaneously). Any PE idle gap before the
    array is warm also resets the HAM clock-gate's 3.4us busy-window
    (PE stays at 1.2GHz), so one late semaphore can cascade into
    +3-5us. Designs that staged the head finely to start the GEMM
    early measured best-case 44.8us but worst-case 50us; this version
    keeps the single-DMA head + a warmup chain sized past worst-case
    visibility (~12.2us), trading ~0.7us of best-case for a ~0.3us
    sigma.
  - Startup: one packed 640KB DMA on the SP ring ([xs0|w0|w1|w2|w3] -
    one completion semaphore gates the first real matmul; sub-256KB
    pieces also pay per-transfer overhead that halves effective wire
    rate). The fp16 bias follows on the ACT ring (needed only by the
    first bias-add, ~2us of slack, PSUM pool gives ~5 tiles of
    slack). The scratch memset runs on DVE (free at ~6.9us; GpSimd is
    busy with framework work until ~7.7us), so the warmup chain
    starts ~0.4us earlier than on GpSimd.
  - PE p-state warmup: ~48 throwaway 128-col matmuls on a [128,128]
    memset scratch tile (one accumulation group - no inter-matmul
    semaphores) keep the PE busy from ~7.5us until the head is
    consumer-visible (~12.3us worst case). 128-col LDW+MM pairs
    stream at 107ns cold / 56ns warm, so once the HAM clock-gate
    fires mid-warmup the remaining chain shrinks and hands off
    earlier - mildly adaptive in the right direction. The real GEMM
    then runs fully ramped at ~216ns per [128x128]@[128x512] fp16
    matmul - the PE roofline - with ~0 stall across all 132 matmuls
    (zero stalls across 12 traced runs).
  - Token slabs (128/128/256 ramp-in, 512 steady, 384/128/128 tail)
    stream through SBUF on the SP ring. The slab pool is deliberately
    only 4 buffers deep: pool back-pressure keeps slab 5+ (not
    needed until ~19us+) OUT of the congested 8-13us startup window,
    cutting chip-wide early wire traffic ~20% (all 8 cores run this
    same program) and with it the semaphore-propagation lag that
    stalls tiles 1-4. (bufs=3 was tried: it eliminated the early
    stalls entirely but over-throttled - late slabs then arrived
    ~1us late at tiles 20/24. bufs=4 keeps ~4.5us of late-slab
    margin.) Per 128-token tile, 4 accumulating matmuls into one
    PSUM bank; DVE adds the bias while moving PSUM->SBUF. Outputs go out on the ACT HWDGE ring except
    the second-to-last slab (idle SP ring). The last tile is computed
    as two half-width accumulation groups in recycled warmup-PSUM
    tiles so the first half's bias-add and store launch before the
    last matmul retires, and the final transfer (SP ring) overlaps
    the second half's add.
  - Operands and output are fp16 (values are O(1); PSUM accumulation
    stays fp32): absmax 2.7e-3 on scale-5.5 outputs (4.9e-4
    relative). fp8 in any arrangement fails the 2e-2 gate (measured
    numerically: e4m3 both operands 3.7e-2, one operand 2.7e-2, even
    ideal e3m4 2.0e-2), so fp16 is the fastest legal dtype.
"""

import os

import numpy as np

N_EXPERTS = 8
D_IN = 512
D_OUT = 512
N_TOKENS = 32768
CAP = 4224  # per-expert token capacity: 33*128; host fallback covers overflow
TOK_SLAB = 512
KC = D_IN // 128  # 4 contraction chunks

# Warmup sizing: starts ~7.15us (tiny DVE memset right after the
# framework preamble) and must END at worst-case head visibility
# (~12.3us). Ending early exposes the run to a PE idle gap that
# resets the HAM busy-window; ending late delays the GEMM 1:1.
# All-128-col warmup: 107ns per LDW+MM pair at the cold clock
# (measured - LDWEIGHTS overlaps the previous MM via the PE's second
# SBUF read port), and the scratch memset is only [128,128] (~150ns
# on DVE vs ~480ns for a [128,512] tile).
WARM128 = int(os.environ.get("KERNEL_WARM128", "48"))


def _slab_schedule():
    head_sizes = [128, 128, 256]
    tail_sizes = [128, 128]
    sizes = list(head_sizes)
    remaining = CAP - sum(head_sizes) - sum(tail_sizes)
    while remaining > 0:
        sizes.append(min(TOK_SLAB, remaining))
        remaining -= sizes[-1]
    sizes.extend(tail_sizes)
    slabs = []
    t0 = 0
    for ts in sizes:
        slabs.append((t0, ts))
        t0 += ts
    assert t0 == CAP
    return slabs


SLABS = _slab_schedule()
Y_FREE = (CAP // 128) * D_OUT  # packed output free size per partition
HEAD_TOK = SLABS[0][1]  # tokens in slab 0 (rides in the head pack)
HEAD_FREE = KC * HEAD_TOK + KC * D_OUT  # [xs0 | w0 | w1 | w2 | w3]

# mode -> (x dtype, w dtype, y dtype); x and w must match (packed DMAs).
MM_DTYPE = os.environ.get("KERNEL_MM_DTYPE", "float16_o16")
_DT_MAP = {
    "float32": ("float32", "float32", "float32"),
    "float32r": ("float32r", "float32r", "float32"),
    "float32r_o16": ("float32r", "float32r", "float16"),
    "bfloat16": ("bfloat16", "bfloat16", "float32"),
    "float16": ("float16", "float16", "float32"),
    "float16_o16": ("float16", "float16", "float16"),
}

_cache = {}


def _build(mm_dtype_name):
    import concourse.bacc as bacc
    import concourse.mybir as mybir
    import concourse.tile as tile

    x_dt_name, w_dt_name, y_dt_name = _DT_MAP[mm_dtype_name]
    assert x_dt_name == w_dt_name
    dt_x = getattr(mybir.dt, x_dt_name)
    dt_y = getattr(mybir.dt, y_dt_name)
    f32 = mybir.dt.float32

    nc = bacc.Bacc("TRN2", target_bir_lowering=False, debug=False, num_devices=N_EXPERTS)
    # Slab-contiguous packed layouts: one contiguous run per partition
    # per slab DMA. head = [xs_slab0 | all four w chunks]; xt's slab-0
    # region is unused (kept so the host packer stays uniform).
    xt = nc.dram_tensor("xt", (128, KC * CAP), dt_x, kind="ExternalInput").ap()
    head = nc.dram_tensor("head", (128, HEAD_FREE), dt_x, kind="ExternalInput").ap()
    b = nc.dram_tensor("b", (128, D_OUT), dt_x, kind="ExternalInput").ap()
    y = nc.dram_tensor("y", (128, Y_FREE), dt_y, kind="ExternalOutput").ap()

    with tile.TileContext(nc) as tc:
        with (
            tc.tile_pool(name="wpool", bufs=1) as wpool,
            tc.tile_pool(name="bias", bufs=1) as bias_pool,
            tc.tile_pool(name="warm", bufs=1) as warm_pool,
            tc.tile_pool(name="xslab", bufs=4) as xpool,
            tc.tile_pool(name="ystage", bufs=8) as ypool,
            tc.tile_pool(name="psum", bufs=6, space="PSUM") as pspool,
            tc.tile_pool(name="wpsum", bufs=2, space="PSUM") as warm_ps_pool,
        ):
            slabs = SLABS

            # Startup DMAs: one packed transfer on the SP ring carries
            # everything the first groups need; bias rides the ACT
            # ring. (Routing the bias via SWDGE was tried and
            # REGRESSED: the SWDGE descriptor rings live on SBUF
            # partitions whose AXI ports also serve SDMA engines,
            # slowing the head/slab stream during startup.)
            head_sb = wpool.tile([128, HEAD_FREE], dt_x, tag="head", name="head_sb")
            b_rep = bias_pool.tile([128, D_OUT], dt_x, tag="brep")
            nc.sync.dma_start(head_sb[:], head[:])
            # Bias rides the ACT ring. (Deferring it onto the SP ring
            # behind xt3 was tried and REGRESSED uniformly +1.6us:
            # under congestion it became visible only ~20us and the
            # 6-deep PSUM pool's bias-add deadline stalled tile 8.)
            nc.scalar.dma_start(b_rep[:], b[:])

            # PE p-state warmup (see module docstring): a stream of
            # 128-col matmuls on a small memset scratch tile, one big
            # accumulation group (no inter-matmul semaphores). (Tile
            # requires a producer for every read tile, and the ~150ns
            # DVE memset finishes before the PE exits the framework
            # preamble anyway.)
            scratch = warm_pool.tile([128, 128], dt_x, tag="scr")
            nc.vector.memset(scratch[:], 0.0)
            wps_a = warm_ps_pool.tile([128, D_OUT], f32, tag="wacc")
            for i in range(WARM128):
                nc.tensor.matmul(
                    wps_a[:, 0:128], scratch[:], scratch[:],
                    start=(i == 0), stop=(i == WARM128 - 1),
                )

            xs0_off = KC * HEAD_TOK
            w_aps = [
                head_sb[:, xs0_off + k * D_OUT : xs0_off + (k + 1) * D_OUT]
                for k in range(KC)
            ]

            def load_x(slab_i):
                t0, ts = slabs[slab_i]
                xs = xpool.tile([128, KC * ts], dt_x, tag="xs")
                nc.sync.dma_start(xs[:], xt[:, KC * t0 : KC * (t0 + ts)])
                return xs

            xs_pending = load_x(1)

            n_slabs = len(slabs)
            for i, (t0, ts) in enumerate(slabs):
                nt = ts // 128
                if i == 0:
                    xs = head_sb[:, 0:xs0_off]
                else:
                    xs = xs_pending[:]
                    if i + 1 < n_slabs:
                        xs_pending = load_x(i + 1)
                ys = ypool.tile([128, nt * D_OUT], dt_y, tag="ys")
                last = i == n_slabs - 1
                o0 = (t0 // 128) * D_OUT
                if last:
                    # Final tile: two half-width accumulation groups (8x
                    # 256-col matmuls, same PE cost) in recycled warmup
                    # PSUM tiles, so the first half's bias-add and store
                    # launch ~0.43us before the last matmul retires and
                    # the final transfer overlaps the second half's add.
                    h = D_OUT // 2
                    ts_off = [k * ts for k in range(KC)]
                    ps_h1 = warm_ps_pool.tile([128, D_OUT], f32, tag="wacc")
                    for k in range(KC):
                        nc.tensor.matmul(
                            ps_h1[:, 0:h],
                            xs[:, ts_off[k] : ts_off[k] + 128],
                            w_aps[k][:, 0:h],
                            start=(k == 0),
                            stop=(k == KC - 1),
                        )
                    nc.vector.tensor_add(
                        ys[:, 0:h], ps_h1[:, 0:h], b_rep[:, 0:h]
                    )
                    nc.scalar.dma_start(y[:, o0 : o0 + h], ys[:, 0:h])
                    ps_h2 = warm_ps_pool.tile([128, D_OUT], f32, tag="wacc")
                    for k in range(KC):
                        nc.tensor.matmul(
                            ps_h2[:, 0:h],
                            xs[:, ts_off[k] : ts_off[k] + 128],
                            w_aps[k][:, h:D_OUT],
                            start=(k == 0),
                            stop=(k == KC - 1),
                        )
                    nc.vector.tensor_add(
                        ys[:, h:D_OUT], ps_h2[:, 0:h], b_rep[:, h:D_OUT]
                    )
                    nc.sync.dma_start(
                        y[:, o0 + h : o0 + D_OUT], ys[:, h:D_OUT]
                    )
                    continue
                for a in range(nt):
                    ps = pspool.tile([128, D_OUT], f32, tag="acc")
                    for k in range(KC):
                        nc.tensor.matmul(
                            ps[:],
                            xs[:, k * ts + a * 128 : k * ts + (a + 1) * 128],
                            w_aps[k],
                            start=(k == 0),
                            stop=(k == KC - 1),
                        )
                    nc.vector.tensor_add(
                        ys[:, a * D_OUT : (a + 1) * D_OUT], ps[:], b_rep[:]
                    )
                if not last:
                    # Outputs ride the ACT HWDGE ring until the input
                    # stream is done; the last few slabs alternate onto
                    # the idle SP ring so the end-of-kernel drain
                    # splits across both ring FIFOs and the final
                    # pieces sit near each queue's head. (Early
                    # outputs must NOT go on Sync: an output DMA
                    # instruction waiting on its bias-add semaphore
                    # blocks later input-slab issues in the engine's
                    # FIFO - measured +2.4us in an earlier variant.)
                    eng = nc.sync if i in (n_slabs - 2, n_slabs - 4) else nc.scalar
                    eng.dma_start(y[:, o0 : o0 + nt * D_OUT], ys[:])
    nc.compile()
    return nc


def _get_nc(mm_dtype_name):
    if mm_dtype_name not in _cache:
        _cache[mm_dtype_name] = _build(mm_dtype_name)
    return _cache[mm_dtype_name]


def kernel(x, index, weight, bias, _trace=False):
    from concourse.bass_utils import run_bass_kernel_spmd

    x = np.ascontiguousarray(np.asarray(x, dtype=np.float32))
    weight = np.ascontiguousarray(np.asarray(weight, dtype=np.float32))
    bias = np.ascontiguousarray(np.asarray(bias, dtype=np.float32))
    idx = np.asarray(index).astype(np.int64, copy=False)

    ids = [np.nonzero(idx == e)[0] for e in range(N_EXPERTS)]

    in_maps = []
    for e in range(N_EXPERTS):
        n_e = min(len(ids[e]), CAP)
        x_e = np.zeros((CAP, D_IN), dtype=np.float32)
        x_e[:n_e] = x[ids[e][:n_e]]
        # Pack slab-major: xt_e[p, KC*t0 + kc*ts + t] = x_e[t0+t, kc*128+p]
        xt_e = np.empty((128, KC * CAP), dtype=np.float32)
        for t0, ts in SLABS:
            blk = x_e[t0 : t0 + ts].reshape(ts, KC, 128)  # [t, kc, p]
            xt_e[:, KC * t0 : KC * (t0 + ts)] = (
                blk.transpose(2, 1, 0).reshape(128, KC * ts)
            )
        w_e = weight[e]
        head_e = np.concatenate(
            [xt_e[:, 0 : KC * HEAD_TOK]]
            + [w_e[k * 128 : (k + 1) * 128, :] for k in range(KC)],
            axis=1,
        )
        in_maps.append(
            {
                "xt": xt_e,
                "head": np.ascontiguousarray(head_e),
                "b": np.ascontiguousarray(
                    np.broadcast_to(bias[e], (128, D_OUT))
                ),
            }
        )

    x_dt_name, _, _ = _DT_MAP[MM_DTYPE]
    cast = {"bfloat16": None, "float16": np.float16, "float32": np.float32,
            "float32r": np.float32}
    ct = cast[x_dt_name]
    if ct is None:
        import ml_dtypes

        ct = ml_dtypes.bfloat16
    in_maps = [
        {
            **m,
            "xt": m["xt"].astype(ct),
            "head": m["head"].astype(ct),
            "b": m["b"].astype(ct),
        }
        for m in in_maps
    ]

    nc = _get_nc(MM_DTYPE)
    res = run_bass_kernel_spmd(
        nc, in_maps, core_ids=list(range(N_EXPERTS)), trace=_trace
    )

    out = np.empty((x.shape[0], D_OUT), dtype=np.float32)
    for e in range(N_EXPERTS):
        n_e = min(len(ids[e]), CAP)
        # Unpack [p, a_global, o] -> token-major [a_global*128+p, o]
        y_pm = res.results[e]["y"].reshape(128, CAP // 128, D_OUT)
        y_e = y_pm.transpose(1, 0, 2).reshape(CAP, D_OUT)
        out[ids[e][:n_e]] = y_e[:n_e].astype(np.float32)
        if len(ids[e]) > CAP:  # capacity overflow: host fallback (correctness net)
            over = ids[e][CAP:]
            out[over] = x[over] @ weight[e] + bias[e]

    if _trace:
        return out, res
    return out
